# revision 41
# baseline (speedup 1.0000x reference)
"""DenseCRF loss kernel for Trainium2 (8 NeuronCores, SPMD).

loss = -(WEIGHT/N) * sum_n sum_k  s_k^T K s_k,   K_ij = exp(-0.5*||f_i-f_j||^2)

with 5-dim pixel features f = [x/100, y/100, g, g, g], g = img*255/15.
The 3 identical gray channels collapse to one feature sqrt(3)*g.

Strategy:
  * K is symmetric -> only compute the lower block-triangle of the [HW,HW]
    affinity at [128,512] tile granularity; off-diagonal-chunk tiles carry
    weight 2 (folded into the segmentation weights).
  * The exp argument -0.5*d2 = f_i.f_j - 0.5|f_i|^2 - 0.5|f_j|^2 is built by a
    single PE matmul over 9 bf16 contraction rows: 3 features + the norm term
    of each side as a triple-bf16 split (hi/mid/lo) against constant-1 rows.
    Because |f~|^2 is computed on host from the *bf16-rounded* features, the
    bf16 input rounding cancels exactly in the quadratic form.
  * ACT evaluates exp PSUM->SBUF (bf16); a second PE matmul contracts each
    tile with the K=2 per-class weights into a [2,512] PSUM stripe; quads of 4
    stripes are copied out via DVE+DMA. Host finishes with a ~1M-madd epilogue.
  * PE array packing: the 4 mm1s of a quad (contract dim 9) run in 4 distinct
    32-row groups concurrently; the 4 mm2s (output dim 2) run in 4 distinct
    32-col groups concurrently -- 4x PE throughput vs naive.

Work: 2 images x 171 quads (+2 dummy) = 344 quads -> 43 quads/core on 8
cores. Single SPMD program; all per-core differences live in packed inputs.
"""

import numpy as np
import ml_dtypes

# ---------------------------------------------------------------- constants
WEIGHT = 2e-9
N_IMG, K_CLS, H, W = 2, 2, 96, 96
HW = H * W                      # 9216
CHUNK = 512                     # column chunk (one PSUM bank of fp32)
NCHUNK = HW // CHUNK            # 18
PBLK = 128                      # row block (PE partition dim)
N_CORES = 8
ACT_GRP = 3                     # tiles exp'd per ACT instruction (3 PSUM banks)
QBLK = 8                        # quads per output-staging block

# quad = (image n, column chunk c, row group g) covering row tiles 4g..4g+3 of
# chunk c; g<c tiles are strictly above the diagonal chunk -> weight 2.
QUADS = [(n, c, g) for n in range(N_IMG) for c in range(NCHUNK) for g in range(c + 1)]
QPC = -(-len(QUADS) // N_CORES)          # 43 quads per core
QUADS_PADDED = QUADS + [None] * (QPC * N_CORES - len(QUADS))
TILES_PC = QPC * 4                       # 172 tiles per core

_bf16 = ml_dtypes.bfloat16

# Final kernel configuration (validated on hardware):
PACK_MM1 = True    # concurrent row-group mm1 packing (4x32-row groups)
PACK_MM2 = True    # concurrent col-group mm2 packing (4x32-col groups)
_PROGRAM = None


# ---------------------------------------------------------------- device code
def _build_program(pack_mm1=True, pack_mm2=True, drain_between=False,
                   chain_pe=True):
    import concourse.bacc as bacc
    import concourse.tile as tile
    from concourse.tile import add_dep_helper
    from concourse import mybir

    nc = bacc.Bacc(None)

    # PE instruction-order chaining: every LDWEIGHTS writes the shared PE
    # weight-cell array, so a foreign ldw scheduled between a pack's ldw and
    # its matmul corrupts in-flight results. Chain matmuls in emission order
    # so the Tile scheduler cannot interleave mm2s into mm1 packs.
    _last_mm = [None]

    def _chain(inst):
        if chain_pe:
            cur = getattr(inst, "ins", inst)
            if _last_mm[0] is not None:
                add_dep_helper(cur, _last_mm[0], sync=False,
                               reason="pe weight-cell order")
            _last_mm[0] = cur
        return inst

    def _mm(*args, **kw):
        return _chain(nc.tensor.matmul(*args, **kw))

    def _drain():
        return _chain(nc.tensor.drain())
    # flhs: row-banded weights. band j (partitions 32j..32j+8) holds the
    # [9,128] lhsT block of quad-tile j at cols i*128. Flat layout (unpacked
    # mm1): all blocks at partitions 0-8, cols t*128.
    flhs_shape = [128, QPC * PBLK] if pack_mm1 else [9, TILES_PC * PBLK]
    flhs_d = nc.dram_tensor("flhs", flhs_shape, mybir.dt.bfloat16,
                            kind="ExternalInput")
    # frhs: [9, QPC*512]; replicated on-device into 4 row bands.
    frhs_d = nc.dram_tensor("frhs", [9, QPC * CHUNK], mybir.dt.bfloat16,
                            kind="ExternalInput")
    wt_d = nc.dram_tensor("wt", [128, TILES_PC * 2], mybir.dt.bfloat16,
                          kind="ExternalInput")
    # M stripes staged in SBUF and written out in blocks of QBLK quads:
    # mout[b, j, k, (i%QBLK)*512 + q] for quad i = QBLK*b + s, tile j, class k.
    NBLK = -(-QPC // QBLK)
    mout_d = nc.dram_tensor("mout", [NBLK, 4, 2, QBLK * CHUNK],
                            mybir.dt.float32, kind="ExternalOutput")

    with tile.TileContext(nc) as tc:
        with (
            tc.tile_pool(name="consts", bufs=1) as consts,
            tc.tile_pool(name="gps", bufs=2, space="PSUM") as gpool,
            tc.tile_pool(name="mps", bufs=2, space="PSUM") as mpool,
            tc.tile_pool(name="esb", bufs=5) as epool,
            tc.tile_pool(name="msb", bufs=2) as mspool,
        ):
            # Chunk the input loads over quad ranges so the first quads'
            # operands land quickly instead of stalling ~15us on full-size
            # band transfers.
            flhs = consts.tile(flhs_shape, mybir.dt.bfloat16)
            frhs = consts.tile([128, QPC * CHUNK], mybir.dt.bfloat16)
            wt = consts.tile([128, TILES_PC * 2], mybir.dt.bfloat16)
            bounds = [0, 6, 16, 28, QPC]
            for k in range(len(bounds) - 1):
                q0, q1 = bounds[k], bounds[k + 1]
                if pack_mm1:
                    nc.sync.dma_start(
                        out=flhs[:, q0 * PBLK:q1 * PBLK],
                        in_=flhs_d[:, q0 * PBLK:q1 * PBLK])
                else:
                    nc.sync.dma_start(
                        out=flhs[:, q0 * 4 * PBLK:q1 * 4 * PBLK],
                        in_=flhs_d[:, q0 * 4 * PBLK:q1 * 4 * PBLK])
                nbands = 4 if pack_mm1 else 1
                for j in range(nbands):
                    nc.sync.dma_start(
                        out=frhs[32 * j:32 * j + 9, q0 * CHUNK:q1 * CHUNK],
                        in_=frhs_d[:, q0 * CHUNK:q1 * CHUNK])
                nc.sync.dma_start(out=wt[:, q0 * 8:q1 * 8],
                                  in_=wt_d[:, q0 * 8:q1 * 8])

            # G/E slot bookkeeping: slot s of group g holds tile t = 3g+s.
            gts, ets, mstage = {}, {}, {}

            def gslot(t):
                g, s = divmod(t, ACT_GRP)
                if g not in gts:
                    gts[g] = gpool.tile([128, ACT_GRP * CHUNK],
                                        mybir.dt.float32, tag="g", name="gt")
                return gts[g][:, s * CHUNK:(s + 1) * CHUNK]

            def eslot(t):
                g, s = divmod(t, ACT_GRP)
                return ets[g][:, s * CHUNK:(s + 1) * CHUNK]

            def mm2_pack(i):
                # ---- mm2: 4 matmuls (output dim 2), col-group-packed or not
                if drain_between:
                    _drain()
                mq = mpool.tile([128, CHUNK], mybir.dt.float32, tag="m",
                                name="mq")
                for j in range(4):
                    t = 4 * i + j
                    pos = 32 * j if pack_mm2 else 0
                    _mm(
                        mq[pos:pos + 2, :],
                        lhsT=wt[:, t * 2:(t + 1) * 2],
                        rhs=eslot(t),
                        start=(True if pack_mm2 else j == 0),
                        stop=(True if pack_mm2 else j == 3),
                        tile_position=(0, pos) if pack_mm2 else None,
                    )
                if drain_between:
                    _drain()
                b, s = divmod(i, QBLK)
                if s == 0:
                    mstage[b] = mspool.tile([128, QBLK * CHUNK],
                                            mybir.dt.float32, tag="ms",
                                            name="ms")
                st = mstage[b]
                nc.vector.tensor_copy(out=st[:, s * CHUNK:(s + 1) * CHUNK],
                                      in_=mq[:])
                if s == QBLK - 1 or i == QPC - 1:
                    n = (s + 1) * CHUNK
                    rows = range(4) if pack_mm2 else range(1)
                    for j in rows:
                        nc.sync.dma_start(out=mout_d[b, j, :, 0:n],
                                          in_=st[32 * j:32 * j + 2, 0:n])

            for i in range(QPC):
                # ---- mm1: 4 row-group-packed matmuls (contract dim 9)
                for j in range(4):
                    t = 4 * i + j
                    if pack_mm1:
                        lslice = flhs[32 * j:32 * j + 9,
                                      i * PBLK:(i + 1) * PBLK]
                        rslice = frhs[32 * j:32 * j + 9,
                                      i * CHUNK:(i + 1) * CHUNK]
                        pos = (32 * j, 0)
                    else:
                        lslice = flhs[0:9, t * PBLK:(t + 1) * PBLK]
                        rslice = frhs[0:9, i * CHUNK:(i + 1) * CHUNK]
                        pos = None
                    _mm(gslot(t), lhsT=lslice, rhs=rslice,
                        start=True, stop=True, tile_position=pos)
                    # ---- exp whenever an ACT group fills (3 tiles)
                    g, s = divmod(t, ACT_GRP)
                    if s == ACT_GRP - 1 or t == TILES_PC - 1:
                        et = epool.tile([128, ACT_GRP * CHUNK],
                                        mybir.dt.bfloat16, tag="e", name="et")
                        n = (s + 1) * CHUNK
                        nc.scalar.activation(
                            out=et[:, :n], in_=gts[g][:, :n],
                            func=mybir.ActivationFunctionType.Exp)
                        ets[g] = et
                # 2-quad lag: quad i-2's E groups all fired during quad i-1,
                # so the mm2 pack never stalls the PE waiting on ACT.
                if i >= 2:
                    mm2_pack(i - 2)
            mm2_pack(QPC - 2)
            mm2_pack(QPC - 1)
    nc.compile()
    return nc


# ---------------------------------------------------------------- host side
def _features(img_flat):
    """img_flat: [HW] f32 in [0,1] -> (L, R) [9, HW] bf16 matmul operands."""
    ys, xs = np.meshgrid(np.arange(H, dtype=np.float32),
                         np.arange(W, dtype=np.float32), indexing="ij")
    fx = (xs.ravel() / np.float32(100.0)).astype(np.float32)
    fy = (ys.ravel() / np.float32(100.0)).astype(np.float32)
    fg = np.float32(np.sqrt(3.0)) * (img_flat.astype(np.float32) * np.float32(17.0))
    f = np.stack([fx, fy, fg], 0).astype(_bf16).astype(np.float32)  # bf16-rounded
    h = (-0.5 * np.sum(f.astype(np.float64) ** 2, axis=0)).astype(np.float32)
    h1 = h.astype(_bf16).astype(np.float32)
    h2 = (h - h1).astype(_bf16).astype(np.float32)
    h3 = ((h - h1) - h2).astype(_bf16).astype(np.float32)
    ones = np.ones((3, HW), np.float32)
    L = np.concatenate([f, h1[None], h2[None], h3[None], ones], 0).astype(_bf16)
    R = np.concatenate([f, ones, h1[None], h2[None], h3[None]], 0).astype(_bf16)
    return L, R


def _pack(images, segmentations, banded_mm1=True):
    Ls, Rs = [], []
    for n in range(N_IMG):
        L, R = _features(images[n].reshape(-1))
        Ls.append(L)
        Rs.append(R)
    S = segmentations.reshape(N_IMG, K_CLS, HW).astype(np.float32)
    in_maps, metas = [], []
    for core in range(N_CORES):
        myq = QUADS_PADDED[core * QPC:(core + 1) * QPC]
        if banded_mm1:
            flhs = np.zeros((128, QPC * PBLK), _bf16)
        else:
            flhs = np.zeros((9, TILES_PC * PBLK), _bf16)
        frhs = np.zeros((9, QPC * CHUNK), _bf16)
        wt = np.zeros((128, TILES_PC * 2), _bf16)
        for i, qd in enumerate(myq):
            if qd is None:
                continue
            n, c, g = qd
            wgt = np.float32(2.0 if g < c else 1.0)
            frhs[:, i * CHUNK:(i + 1) * CHUNK] = Rs[n][:, c * CHUNK:(c + 1) * CHUNK]
            for j in range(4):
                r = 4 * g + j
                t = 4 * i + j
                blk = Ls[n][:, r * PBLK:(r + 1) * PBLK]
                if banded_mm1:
                    flhs[32 * j:32 * j + 9, i * PBLK:(i + 1) * PBLK] = blk
                else:
                    flhs[:, t * PBLK:(t + 1) * PBLK] = blk
                wt[:, t * 2:(t + 1) * 2] = (
                    wgt * S[n][:, r * PBLK:(r + 1) * PBLK].T).astype(_bf16)
        in_maps.append({"flhs": flhs, "frhs": frhs, "wt": wt})
        metas.append(myq)
    return in_maps, metas, S


def _reduce(results, metas, S):
    total = np.float64(0.0)
    for core in range(N_CORES):
        M = np.asarray(results[core]["mout"]).astype(np.float64)
        for i, qd in enumerate(metas[core]):
            if qd is None:
                continue
            n, c, _g = qd
            V = S[n][:, c * CHUNK:(c + 1) * CHUNK].astype(np.float64)  # [2,512]
            b, s = divmod(i, QBLK)
            Mi = M[b, :, :, s * CHUNK:(s + 1) * CHUNK]  # [4,2,512]
            if PACK_MM2:
                total += float(np.sum(Mi * V[None]))
            else:
                total += float(np.sum(Mi[0] * V))  # stripe 0 = quad sum
    return np.asarray([-WEIGHT * total / N_IMG], dtype=np.float32)


def run(images, segmentations, trace=False, tmpdir=None):
    """Run on hardware; returns (loss[1] f32, BassKernelResults)."""
    from concourse.bass_utils import run_bass_kernel_spmd

    global _PROGRAM
    if _PROGRAM is None:
        _PROGRAM = _build_program(pack_mm1=PACK_MM1, pack_mm2=PACK_MM2)
    in_maps, metas, S = _pack(np.asarray(images), np.asarray(segmentations),
                              banded_mm1=PACK_MM1)
    res = run_bass_kernel_spmd(_PROGRAM, in_maps, core_ids=list(range(N_CORES)),
                               trace=trace, tmpdir=tmpdir)
    return _reduce(res.results, metas, S), res


def kernel(images, segmentations):
    out, _ = run(images, segmentations)
    return out
